# revision 19
# baseline (speedup 1.0000x reference)
"""Trainium2 Bass kernel for a 4-layer LSTM-style stack with local+global logits.

Computation (per example row x of the [16384, 512] input):
    h0 = 0, c0 = 0
    for i in 1..4:
        z  = [x, h_{i-1}] @ W{f,i,o,c} + b        (4 gates, K = 1024)
        c  = tanh(z_c) * sigmoid(z_i) + sigmoid(z_f) * c
        h  = sigmoid(z_o) * tanh(c)
        local_i = h @ Wl_i + bl_i
    global = [x, h4] @ Wg + bg
Returns (concat(local_1..4) [16384, 960], global [16384, 960]).

Strategy (v4):
  - Data-parallel over 8 cores: 2048 rows each, weights replicated.
  - Z = x @ W_top + b computed once per example (bf16), stored scaled x128
    as one [128, 16, 512] tile per quarter (column order t-major: of = t*4+g
    so each hid-tile t's four gates are contiguous).
  - Layers 2-4 hidden-state matmuls run in fp8 e4m3 with
    perf_mode=DoubleRow (K=256 per instruction, 2 fp8 weights per PE cell,
    ~2x bf16 FLOP rate, measured 216 ns per K=256/FD=512 matmul).
    W_bot is prescaled x128 into fp8; h is cast to fp8 unscaled.
    Gates = act((ps + z128)/128) via the activation input scale.
  - Gate phases are Vector-engine-bound (PSUM+z adds); PE work from other
    pipeline stages (Z of later quarters, locals, globals) is emitted
    BETWEEN gate t-blocks so the PSUM ring rotates filler matmuls into the
    windows where the PE would otherwise stall on PSUM drain.
  - Logits stay bf16; eviction via Scalar-engine Copy to bf16 outputs
    (harness-compared at 2e-2 rel-l2; bias added on host, zero here).
  - Elementwise/activation ops are fused across hid-tiles (strided APs).
"""

import os
import sys

import numpy as np

for _p in ("/opt/trn_rl_repo", "/root/.axon_site/_ro/trn_rl_repo"):
    if os.path.isdir(_p) and _p not in sys.path:
        sys.path.insert(0, _p)

import ml_dtypes

import concourse.bass as bass
import concourse.tile as tile
from concourse import bacc, mybir
from concourse.bass_utils import run_bass_kernel_spmd

BF16 = mybir.dt.bfloat16
F32 = mybir.dt.float32
FP8 = mybir.dt.float8e4
AF = mybir.ActivationFunctionType
ALU = mybir.AluOpType
DR = mybir.MatmulPerfMode.DoubleRow

N_CORES = 8
N = 16384
K = 512                  # input features
U = 512                  # hidden units
MC = N // N_CORES        # 2048 rows per core
NQ = 4                   # quarters per core
EXQ = MC // NQ           # 512 examples per quarter
NCLS = [64, 128, 256, 512]
OFFS = [0, 64, 192, 448]
TOT = 960
WS = 128.0               # fp8 weight prescale and z storage scale

IDZ_LAYERS = set()       # layers whose Z-add streams through the PE

LAST_RESULT = None       # BassKernelResults of the most recent run (for test.py)


def _build_program():
    nc = bacc.Bacc("TRN2", target_bir_lowering=False, debug=False)

    xt_d = nc.dram_tensor("xt", [K, MC], BF16, kind="ExternalInput")
    wtop_d = nc.dram_tensor("wtop", [K, 4 * U], BF16, kind="ExternalInput")
    w8a_d = nc.dram_tensor("w8a", [128, 2, 4 * U], FP8, kind="ExternalInput")
    w8b_d = nc.dram_tensor("w8b", [128, 2, 4 * U], FP8, kind="ExternalInput")
    wl_d = nc.dram_tensor("wl", [U, TOT], BF16, kind="ExternalInput")
    wg_d = nc.dram_tensor("wg", [K + U, TOT], BF16, kind="ExternalInput")
    bg128_d = nc.dram_tensor("bg128", [128, 16], F32, kind="ExternalInput")
    ident_d = nc.dram_tensor("ident", [128, 128], BF16, kind="ExternalInput")
    oloc_d = nc.dram_tensor("oloc", [MC, TOT], BF16, kind="ExternalOutput")
    oglb_d = nc.dram_tensor("oglb", [MC, TOT], BF16, kind="ExternalOutput")

    with tile.TileContext(nc) as tc:
        with (
            tc.tile_pool(name="wpool", bufs=1) as wpool,
            tc.tile_pool(name="xpool", bufs=4) as xpool,
            tc.tile_pool(name="zpool", bufs=3) as zpool,
            tc.tile_pool(name="prepool", bufs=1) as prepool,
            tc.tile_pool(name="hpool", bufs=2) as hpool,
            tc.tile_pool(name="cpool", bufs=2) as cpool,
            tc.tile_pool(name="h8pool", bufs=2) as h8pool,
            tc.tile_pool(name="ttp", bufs=1) as ttp,
            tc.tile_pool(name="tcp", bufs=1) as tcp,
            tc.tile_pool(name="l1p", bufs=1) as l1p,
            tc.tile_pool(name="lop", bufs=1) as lop,
            tc.tile_pool(name="glop", bufs=1) as glop,
            tc.tile_pool(name="gpsum", bufs=2, space="PSUM") as gpsum,
        ):
            # ---- resident weights (DMA order = first-use order) ---------
            xs_pre = {}
            tiles = []
            wtop_sb = []
            for kt in range(4):
                t = xpool.tile([128, EXQ], BF16, tag=f"x{kt}")
                nc.sync.dma_start(t[:], xt_d[kt * 128:(kt + 1) * 128, 0:EXQ])
                tiles.append(t)
                w = wpool.tile([128, 4 * U], BF16, tag=f"wt{kt}")
                nc.sync.dma_start(w[:], wtop_d[kt * 128:(kt + 1) * 128, :])
                wtop_sb.append(w)
            xs_pre[0] = tiles
            bg128_sb = wpool.tile([128, 16], F32, tag="bg128")
            nc.sync.dma_start(bg128_sb[:], bg128_d[:])
            tiles = []
            for kt in range(4):
                t = xpool.tile([128, EXQ], BF16, tag=f"x{kt}")
                nc.sync.dma_start(
                    t[:], xt_d[kt * 128:(kt + 1) * 128, EXQ:2 * EXQ])
                tiles.append(t)
            xs_pre[1] = tiles
            ident_sb = wpool.tile([128, 128], BF16, tag="ident")
            nc.sync.dma_start(ident_sb[:], ident_d[:])
            w8_sb = []
            for j, d in enumerate((w8a_d, w8b_d)):
                t = wpool.tile([128, 2, 4 * U], FP8, tag=f"w8{j}")
                nc.sync.dma_start(t[:], d[:])
                w8_sb.append(t)
            wl_sb = []
            for kt in range(4):
                t = wpool.tile([128, TOT], BF16, tag=f"wl{kt}")
                nc.sync.dma_start(t[:], wl_d[kt * 128:(kt + 1) * 128, :])
                wl_sb.append(t)
            wg_sb = []
            for kt in range(8):
                t = wpool.tile([128, TOT], BF16, tag=f"wg{kt}")
                nc.sync.dma_start(t[:], wg_d[kt * 128:(kt + 1) * 128, :])
                wg_sb.append(t)

            # per-quarter live state
            xs = [None] * NQ     # 4 x [128, EXQ] bf16 X^T tiles
            zs = [None] * NQ     # [128, 16, 512] bf16 tile: z*128, of = t*4+g
            hs = [None] * NQ     # 2 x [128, 2, EXQ] bf16 (j-pairs)
            cs = [None] * NQ     # 2 x [128, 2, EXQ] bf16
            h8s = [None] * NQ    # 2 x [128, 2, EXQ] fp8

            def stage_x(q):
                """DMA this quarter's x tiles."""
                if q in xs_pre:
                    xs[q] = xs_pre.pop(q)
                else:
                    xs[q] = []
                    for kt in range(4):
                        t = xpool.tile([128, EXQ], BF16, tag=f"x{kt}")
                        nc.sync.dma_start(
                            t[:], xt_d[kt * 128:(kt + 1) * 128,
                                       q * EXQ:(q + 1) * EXQ])
                        xs[q].append(t)

            def z_chunk(q, og):
                """One of-group (4 of-tiles) of Z: matmul + fused evict.
                z128 = 128*(x @ W_top) [+ 128*b per-of when bias nonzero]."""
                zq = zs[q]
                ps = gpsum.tile([128, 4, EXQ], F32, tag="gp4", name="ps")
                for i in range(4):
                    of = og * 4 + i
                    for kt in range(4):
                        nc.tensor.matmul(
                            ps[:, i, :],
                            wtop_sb[kt][:, of * 128:(of + 1) * 128],
                            xs[q][kt][:], start=(kt == 0), stop=(kt == 3))
                nc.vector.tensor_scalar(
                    zq[:, og * 4:(og + 1) * 4, :], ps[:], WS, None, ALU.mult)

            def stage_z_alloc(q):
                zq = zpool.tile([128, 16, EXQ], BF16, tag="z", name="z")
                zs[q] = zq

            def cand_block(q, pre, j, first, want_h8=False):
                """c/h/h8 update for hid-pair j from gate tile `pre`."""
                a = q % 2
                b0 = 8 * j
                f_ap = pre[:, b0 + 0:b0 + 8:4, :]
                ig_ap = pre[:, b0 + 1:b0 + 8:4, :]
                og_ap = pre[:, b0 + 2:b0 + 8:4, :]
                ch_ap = pre[:, b0 + 3:b0 + 8:4, :]
                cn = cpool.tile([128, 2, EXQ], BF16, tag=f"c{a}{j}",
                                name="cn")
                if first:
                    nc.vector.tensor_mul(cn[:], ig_ap, ch_ap)
                else:
                    t1 = ttp.tile([128, 2, EXQ], BF16, tag="t1", name="t1")
                    nc.vector.tensor_mul(t1[:], ig_ap, ch_ap)
                    t2 = ttp.tile([128, 2, EXQ], BF16, tag="t2", name="t2")
                    nc.vector.tensor_mul(t2[:], f_ap, cs[q][j][:])
                    nc.vector.tensor_add(cn[:], t1[:], t2[:])
                cs[q][j] = cn
                tc_t = tcp.tile([128, 2, EXQ], BF16, tag="tc", name="tc")
                nc.scalar.activation(tc_t[:], cn[:], AF.Tanh)
                if want_h8:
                    h8 = h8pool.tile([128, 2, EXQ], FP8,
                                     tag=f"h8{q % 2}{j}", name="h8")
                    nc.vector.tensor_mul(h8[:], og_ap, tc_t[:])
                    h8s[q][j] = h8
                hn = hpool.tile([128, 2, EXQ], BF16, tag=f"h{a}{j}",
                                name="hn")
                nc.vector.tensor_mul(hn[:], og_ap, tc_t[:])
                hs[q][j] = hn
                return hn

            def stage_l1(q):
                """Layer 1: h0 = 0, gates straight from z (no f gate)."""
                hs[q] = [None, None]
                cs[q] = [None, None]
                h8s[q] = [None, None]
                zq = zs[q]
                for j in range(2):
                    b0 = 8 * j
                    gi = l1p.tile([128, 2, EXQ], BF16, tag="gi", name="gi")
                    nc.scalar.activation(
                        gi[:], zq[:, b0 + 1:b0 + 8:4, :], AF.Sigmoid,
                        scale=1.0 / WS)
                    go = l1p.tile([128, 2, EXQ], BF16, tag="go", name="go")
                    nc.scalar.activation(
                        go[:], zq[:, b0 + 2:b0 + 8:4, :], AF.Sigmoid,
                        scale=1.0 / WS)
                    ch = l1p.tile([128, 2, EXQ], BF16, tag="ch", name="ch")
                    nc.scalar.activation(
                        ch[:], zq[:, b0 + 3:b0 + 8:4, :], AF.Tanh,
                        scale=1.0 / WS)
                    a = q % 2
                    cn = cpool.tile([128, 2, EXQ], BF16, tag=f"c{a}{j}",
                                    name="cn")
                    nc.vector.tensor_mul(cn[:], gi[:], ch[:])
                    cs[q][j] = cn
                    tc_t = tcp.tile([128, 2, EXQ], BF16, tag="tc", name="tc")
                    nc.scalar.activation(tc_t[:], cn[:], AF.Tanh)
                    h8 = h8pool.tile([128, 2, EXQ], FP8,
                                     tag=f"h8{q % 2}{j}", name="h8")
                    nc.vector.tensor_mul(h8[:], go[:], tc_t[:])
                    h8s[q][j] = h8
                    hn = hpool.tile([128, 2, EXQ], BF16, tag=f"h{a}{j}",
                                    name="hn")
                    nc.vector.tensor_mul(hn[:], go[:], tc_t[:])
                    hs[q][j] = hn

            def h_slice(h_prev, kt, e):
                j, i = kt // 2, kt % 2
                return h_prev[j][:, i, e * 128:(e + 1) * 128]

            def locals_chunk(q, layer, h_prev):
                """local_i = h_i @ Wl_i (+bias on host), natural layout."""
                off, ncl = OFFS[layer], NCLS[layer]
                ps = gpsum.tile([128, 4, EXQ], F32, tag="gp4", name="ps")
                for e in range(4):
                    for kt in range(4):
                        nc.tensor.matmul(
                            ps[:, e, 0:ncl],
                            h_slice(h_prev, kt, e),
                            wl_sb[kt][:, off:off + ncl],
                            start=(kt == 0), stop=(kt == 3))
                lo = lop.tile([128, 4, EXQ], BF16, tag="lo", name="lo")
                nc.scalar.activation(
                    lo[:, :, 0:ncl], ps[:, :, 0:ncl], AF.Copy)
                for e in range(4):
                    r0 = q * EXQ + e * 128
                    nc.sync.dma_start(
                        oloc_d[r0:r0 + 128, off:off + ncl], lo[:, e, 0:ncl])

            def global_chunk(q, p, h_prev):
                """globals for examples pair p (2 e-tiles) of quarter q."""
                ps = gpsum.tile([128, 4, EXQ], F32, tag="gp4", name="ps")
                for ie in range(2):
                    e = 2 * p + ie
                    for s, w in ((0, 512), (1, 448)):
                        out = ps[:, ie * 2 + s, 0:w]
                        for kt in range(8):
                            if kt < 4:
                                st = xs[q][kt][:, e * 128:(e + 1) * 128]
                            else:
                                st = h_slice(h_prev, kt - 4, e)
                            nc.tensor.matmul(
                                out, st,
                                wg_sb[kt][:, s * 512:s * 512 + w],
                                start=(kt == 0), stop=(kt == 7))
                gt = glop.tile([128, 4, EXQ], BF16, tag="glo", name="gt")
                nc.scalar.activation(gt[:], ps[:], AF.Copy)
                for ie in range(2):
                    e = 2 * p + ie
                    r0 = q * EXQ + e * 128
                    nc.sync.dma_start(
                        oglb_d[r0:r0 + 128, 0:512], gt[:, ie * 2, :])
                    nc.sync.dma_start(
                        oglb_d[r0:r0 + 128, 512:960], gt[:, ie * 2 + 1, 0:448])

            def stage_pair(qa, qb, layer, fillers=()):
                """One gate layer (2..4) for two quarters, interleaved, with
                PE filler chunks emitted between t-blocks."""
                fillers = list(fillers)
                idz = layer in IDZ_LAYERS
                h8a, h8b = h8s[qa], h8s[qb]
                hs[qa] = [None, None]
                hs[qb] = [None, None]
                h8s[qa] = [None, None]
                h8s[qb] = [None, None]
                pres = {}
                for q in (qa, qb):
                    pres[q] = prepool.tile(
                        [128, 16, EXQ], BF16, tag=f"pre{q % 2}",
                        name=f"pre{q % 2}")
                for t in range(4):
                    pss = {}
                    for q in (qa, qb):
                        pss[q] = gpsum.tile(
                            [128, 4, EXQ], F32, tag="gp4", name="ps")
                    if idz:
                        for q in (qa, qb):
                            for g in range(4):
                                nc.tensor.matmul(
                                    pss[q][:, g, :], ident_sb[:],
                                    zs[q][:, t * 4 + g, :],
                                    start=True, stop=False)
                    # j=0 matmuls for every gate first: h8[j1] of the
                    # previous layer is produced late by its last cand
                    # chain, and the in-order PE queue must not block on it
                    for j in range(2):
                        for g in range(4):
                            w8ap = w8_sb[j][:, :, (t * 4 + g) * 128:
                                            (t * 4 + g + 1) * 128]
                            for q, h8p in ((qa, h8a), (qb, h8b)):
                                nc.tensor.matmul(
                                    pss[q][:, g, :], w8ap, h8p[j][:],
                                    start=(j == 0 and not idz),
                                    stop=(j == 1), perf_mode=DR)
                    for q in (qa, qb):
                        ps, pre = pss[q], pres[q]
                        if idz:
                            nc.scalar.activation(
                                pre[:, t * 4:t * 4 + 3, :], ps[:, 0:3, :],
                                AF.Sigmoid, scale=1.0 / WS)
                            nc.scalar.activation(
                                pre[:, t * 4 + 3, :], ps[:, 3, :],
                                AF.Tanh, scale=1.0 / WS)
                        else:
                            nc.vector.tensor_tensor(
                                pre[:, t * 4:(t + 1) * 4, :], ps[:],
                                zs[q][:, t * 4:(t + 1) * 4, :], ALU.add)
                            nc.scalar.activation(
                                pre[:, t * 4:t * 4 + 3, :],
                                pre[:, t * 4:t * 4 + 3, :],
                                AF.Sigmoid, scale=1.0 / WS)
                            nc.scalar.activation(
                                pre[:, t * 4 + 3, :], pre[:, t * 4 + 3, :],
                                AF.Tanh, scale=1.0 / WS)
                    if t % 2 == 1:
                        j = t // 2
                        for q in (qa, qb):
                            cand_block(q, pres[q], j, first=False,
                                       want_h8=(layer < 4))
                    # inject one PE filler chunk per t-block
                    if fillers:
                        fillers.pop(0)()
                for f in fillers:
                    f()

            # ---- schedule ----------------------------------------------
            # Pipeline: pair (0,1) gate layers carry Z(2)/Z(3)/locals as
            # PE fillers; pair (2,3) gate layers carry pair-A globals.
            stage_x(0)
            stage_x(1)
            stage_z_alloc(0)
            stage_z_alloc(1)
            for og in range(4):
                z_chunk(0, og)
            for og in range(4):
                z_chunk(1, og)
            stage_l1(0)
            stage_x(2)
            stage_z_alloc(2)
            z_chunk(2, 0)
            z_chunk(2, 1)
            stage_l1(1)
            h1 = {q: hs[q] for q in (0, 1)}
            stage_pair(0, 1, 2, fillers=[
                lambda: z_chunk(2, 2),
                lambda: z_chunk(2, 3),
                lambda: locals_chunk(0, 0, h1[0]),
                lambda: locals_chunk(1, 0, h1[1]),
            ])
            stage_x(3)
            h2 = {q: hs[q] for q in (0, 1)}
            stage_pair(0, 1, 3, fillers=[
                lambda: locals_chunk(0, 1, h2[0]),
                lambda: locals_chunk(1, 1, h2[1]),
            ])
            h3 = {q: hs[q] for q in (0, 1)}
            stage_pair(0, 1, 4, fillers=[
                lambda: locals_chunk(0, 2, h3[0]),
                lambda: locals_chunk(1, 2, h3[1]),
            ])
            # z(3) lands in z(0)'s slot: safe only after P4's pre-adds
            stage_z_alloc(3)
            for og in range(4):
                z_chunk(3, og)
            h4 = {q: hs[q] for q in (0, 1)}
            locals_chunk(0, 3, h4[0])
            stage_l1(2)
            global_chunk(0, 0, h4[0])
            global_chunk(0, 1, h4[0])
            locals_chunk(1, 3, h4[1])
            stage_l1(3)
            global_chunk(1, 0, h4[1])
            global_chunk(1, 1, h4[1])
            h1b = {q: hs[q] for q in (2, 3)}
            stage_pair(2, 3, 2, fillers=[
                lambda: locals_chunk(2, 0, h1b[2]),
                lambda: locals_chunk(3, 0, h1b[3]),
            ])
            h2b = {q: hs[q] for q in (2, 3)}
            stage_pair(2, 3, 3, fillers=[
                lambda: locals_chunk(2, 1, h2b[2]),
                lambda: locals_chunk(3, 1, h2b[3]),
            ])
            h3b = {q: hs[q] for q in (2, 3)}
            stage_pair(2, 3, 4, fillers=[
                lambda: locals_chunk(2, 2, h3b[2]),
                lambda: locals_chunk(3, 2, h3b[3]),
            ])
            for q in (2, 3):
                locals_chunk(q, 3, hs[q])
                global_chunk(q, 0, hs[q])
                global_chunk(q, 1, hs[q])

    nc.compile()
    return nc


_PROGRAM = None


def _get_program():
    global _PROGRAM
    if _PROGRAM is None:
        _PROGRAM = _build_program()
    return _PROGRAM


def kernel(inputs, Wf, bf, Wi, bi, Wo, bo, Wc, bc,
           Wl0, bl0, Wl1, bl1, Wl2, bl2, Wl3, bl3, Wg, bg):
    global LAST_RESULT
    bf16 = ml_dtypes.bfloat16
    fp8 = ml_dtypes.float8_e4m3

    inputs = np.ascontiguousarray(np.asarray(inputs, dtype=np.float32))
    xt_all = inputs.T.astype(bf16)                    # [512, 16384]
    wcat = np.concatenate(
        [np.asarray(w, np.float32) for w in (Wf, Wi, Wo, Wc)], axis=1)
    bcat = np.concatenate(
        [np.asarray(b, np.float32) for b in (bf, bi, bo, bc)])  # [2048]
    # gate biases are zero in this problem; the on-device Z path assumes so
    # (they would otherwise need the per-of bias variant of the Z eviction)
    assert not np.any(bcat), "nonzero gate biases unsupported by this kernel"

    # t-major column permutation: new col (t*4+g)*128+m <- old g*512+t*128+m
    P = np.empty(2048, np.int64)
    for t in range(4):
        for g in range(4):
            P[(t * 4 + g) * 128:(t * 4 + g + 1) * 128] = np.arange(
                g * 512 + t * 128, g * 512 + t * 128 + 128)
    wcat_p = wcat[:, P]
    bcat_p = bcat[P]

    wtop = np.ascontiguousarray(wcat_p[:512]).astype(bf16)      # [512, 2048]
    wbot = wcat_p[512:]                                          # [512, 2048]
    w8 = []
    for j in range(2):
        a = np.empty((128, 2, 2048), np.float32)
        for i in range(2):
            a[:, i, :] = wbot[128 * (2 * j + i):128 * (2 * j + i + 1), :]
        w8.append(np.ascontiguousarray((a * WS).astype(fp8)))
    bg128 = np.ascontiguousarray(
        (WS * bcat_p).reshape(16, 128).T.astype(np.float32))     # [128, 16]
    wl = np.concatenate(
        [np.asarray(w, np.float32) for w in (Wl0, Wl1, Wl2, Wl3)],
        axis=1).astype(bf16)                          # [512, 960]
    wg = np.asarray(Wg, np.float32).astype(bf16)      # [1024, 960]
    ident = np.eye(128, dtype=np.float32).astype(bf16)

    in_maps = []
    for c in range(N_CORES):
        in_maps.append({
            "xt": np.ascontiguousarray(xt_all[:, c * MC:(c + 1) * MC]),
            "wtop": wtop, "w8a": w8[0], "w8b": w8[1],
            "wl": wl, "wg": wg, "bg128": bg128, "ident": ident,
        })

    nc = _get_program()
    trace = os.environ.get("BASS_KERNEL_TRACE", "0") == "1"
    tmpdir = os.environ.get("BASS_KERNEL_TMPDIR") or None
    res = run_bass_kernel_spmd(
        nc, in_maps, list(range(N_CORES)), trace=trace, tmpdir=tmpdir)
    LAST_RESULT = res

    loc = np.concatenate(
        [r["oloc"] for r in res.results], axis=0).astype(np.float32)
    glb = np.concatenate(
        [r["oglb"] for r in res.results], axis=0).astype(np.float32)
    # logit biases applied host-side (zero in this problem, kept general)
    blcat = np.concatenate(
        [np.asarray(b, np.float32) for b in (bl0, bl1, bl2, bl3)])
    if np.any(blcat):
        loc = loc + blcat
    bg_np = np.asarray(bg, np.float32)
    if np.any(bg_np):
        glb = glb + bg_np
    return loc, glb
